# revision 5
# baseline (speedup 1.0000x reference)
"""Entmax-1.5 (bisection reference) kernel for Trainium2, 8-core data parallel.

The reference runs 50 bisection iterations on tau with bracket
[min(xs)-1, max(xs)=0], xs = x - rowmax(x), z = 0.5*xs,
y = clip(z - tau, 0)^2, constraint = sum(y) - 1, and the update
  tmin = where(constraint < 0, tau, tmin)
  tmax = where(constraint > 0, tau, tmax)
For any row of width N >= 5 the first midpoint tau_1 = (min(xs)-1)/2
satisfies z_i - tau_1 = (xs_i - min(xs) + 1)/2 >= 1/2 for every i, so
constraint >= N/4 - 1 > 0 at tau_1 and at every later (smaller) tau.
Only tmax ever updates, and the f32 halving sequence collapses onto
tmin = min(xs) - 1 within ~30 iterations. Hence the reference equals

    w_i = (0.5*x_i + b)^2,  b = 0.5*rowmax(x) - rowmin(x) + 1
    out = w / (rowsum(w) + 1e-12)

(verified numerically: 5e-7 elementwise relative vs the 50-iter loop).

Kernel per core (512 rows x 32000 cols f32):
  per 128-row chunk, 4 column tiles of 8000:
    DMA in -> DVE rowmax+rowmin per tile -> combine, b = 0.5*max - min + 1
    ACT Square pass (scale=0.5, bias=b) in-place with accumulated rowsum
    DVE reciprocal of (S + 1e-12); ACT Copy-with-scale pass in-place
    DMA out
"""

import numpy as np

N_CORES = 8
ROWS, COLS = 4096, 32000
RPC = ROWS // N_CORES  # rows per core
P = 128  # SBUF partitions
WTILE = 8000  # column tile width
XBUFS = 5  # x-tile slots (each 128 x WTILE f32)


def _build(rows, cols, wtile, xbufs=XBUFS):
    import concourse.bass as bass
    import concourse.tile as tile
    from concourse import bacc, mybir

    f32 = mybir.dt.float32
    AX = mybir.AxisListType.X
    ALU = mybir.AluOpType
    ACTF = mybir.ActivationFunctionType

    assert rows % P == 0 and cols % wtile == 0
    nchunks = rows // P
    ntiles = cols // wtile

    # Bacc (not raw Bass): its compile() runs generate_event_semaphores,
    # which splits multi-wait sync_info to satisfy the TRN2 1-wait/inst limit.
    nc = bacc.Bacc()
    x = nc.declare_dram_parameter("x", [rows, cols], f32, isOutput=False)
    out = nc.declare_dram_parameter("out", [rows, cols], f32, isOutput=True)

    with tile.TileContext(nc) as tc:
        with (
            tc.tile_pool(name="xp", bufs=xbufs) as xp,
            tc.tile_pool(name="sp", bufs=3) as sp,
        ):
            for c in range(nchunks):
                r0 = c * P
                xt = [
                    xp.tile([P, wtile], f32, tag="xt", name=f"xt{c}_{j}")
                    for j in range(ntiles)
                ]
                mx = sp.tile([P, ntiles], f32, tag="mx")
                mn = sp.tile([P, ntiles], f32, tag="mn")
                s = sp.tile([P, ntiles], f32, tag="s")
                xmax = sp.tile([P, 1], f32, tag="xmax")
                xmin = sp.tile([P, 1], f32, tag="xmin")
                bias0 = sp.tile([P, 1], f32, tag="bias0")
                ssum = sp.tile([P, 1], f32, tag="ssum")
                rcp = sp.tile([P, 1], f32, tag="rcp")

                for j in range(ntiles):
                    nc.sync.dma_start(
                        out=xt[j], in_=x[r0 : r0 + P, j * wtile : (j + 1) * wtile]
                    )
                for j in range(ntiles):
                    nc.vector.tensor_reduce(
                        out=mx[:, j : j + 1], in_=xt[j], axis=AX, op=ALU.max
                    )
                    nc.vector.tensor_reduce(
                        out=mn[:, j : j + 1], in_=xt[j], axis=AX, op=ALU.min
                    )
                nc.vector.tensor_reduce(out=xmax, in_=mx, axis=AX, op=ALU.max)
                nc.vector.tensor_reduce(out=xmin, in_=mn, axis=AX, op=ALU.min)
                # bias0 = 0.5*xmax + 1 - xmin
                nc.vector.tensor_scalar(
                    out=bias0,
                    in0=xmax,
                    scalar1=0.5,
                    scalar2=1.0,
                    op0=ALU.mult,
                    op1=ALU.add,
                )
                nc.vector.tensor_tensor(
                    out=bias0, in0=bias0, in1=xmin, op=ALU.subtract
                )
                # w = (0.5*x + bias0)^2 in place, with per-row sum
                for j in range(ntiles):
                    nc.scalar.activation(
                        out=xt[j],
                        in_=xt[j],
                        func=ACTF.Square,
                        bias=bias0,
                        scale=0.5,
                        accum_out=s[:, j : j + 1],
                    )
                nc.vector.tensor_reduce(out=ssum, in_=s, axis=AX, op=ALU.add)
                nc.vector.tensor_scalar(
                    out=ssum, in0=ssum, scalar1=1e-12, scalar2=None, op0=ALU.add
                )
                nc.vector.reciprocal(out=rcp, in_=ssum)
                # out = w * (1/S) in place, then store
                for j in range(ntiles):
                    nc.scalar.activation(
                        out=xt[j], in_=xt[j], func=ACTF.Copy, bias=0.0, scale=rcp
                    )
                    nc.sync.dma_start(
                        out=out[r0 : r0 + P, j * wtile : (j + 1) * wtile], in_=xt[j]
                    )
    # Run Bacc passes (register allocation + the 1-wait/inst sync split).
    # run_bass_via_pjrt serializes nc as-is and never finalizes prebuilt
    # modules; without this walrus crashes on unallocated virtual registers.
    nc.finalize()
    return nc


def kernel(x: np.ndarray) -> np.ndarray:
    from concourse.bass_utils import run_bass_kernel_spmd

    x = np.ascontiguousarray(x, dtype=np.float32)
    assert x.shape == (ROWS, COLS)
    nc = _build(RPC, COLS, WTILE)
    in_maps = [{"x": x[i * RPC : (i + 1) * RPC]} for i in range(N_CORES)]
    res = run_bass_kernel_spmd(nc, in_maps, list(range(N_CORES)))
    return np.concatenate([r["out"] for r in res.results], axis=0)
